# revision 13
# baseline (speedup 1.0000x reference)
"""Trainium2 Bass kernel for NoTPAttention — fp8 q/k projection variant.

(dense transformer block:
fused QKV projection -> multi-head attention -> output projection).

Sharding (8 NeuronCores): core c handles batch b = c // 4 and the 4 heads
g = 4*(c % 4) .. 4*(c % 4)+3 (head-parallel tensor parallelism).  Each core
computes its heads' partial out-projection [S, H] in bf16; the host sums the
4 partials per batch in fp32 and adds the (folded) biases.

Numerics: all matmuls run in bf16 with fp32 PSUM accumulation.  Softmax is
computed without max-subtraction (scores are bounded, |s| < ~3.5) with the
normalization deferred to the attention *output*:
    attnT[d, q] = (sum_k v[k, d] * exp(sT[k, q])) / z[q],  z = sum_k exp
The denominator is NOT a full ones-matmul over every e-tile (that would cost
as much tensor time as the pv matmul).  Instead the DVE pre-reduces the 16
key-tiles of e elementwise (a 6-instruction bf16 add-ladder, cheap on the
otherwise idle vector engine), and a SINGLE ones-matmul per chunk
partition-reduces the [128, QC] ladder result, landing z already broadcast
across partitions.  This cuts the z tensor cost 16x (~51us/core).
The v-bias is dropped in-kernel: after normalization it contributes exactly
b_v to every row, so the host folds w_out @ b_v into the output bias.

Layout notes: qT/kT/attnT live as [128 (head-dim), head, seq] so every
matmul contracts over a full 128-partition tile with no transposes anywhere.
The qkv weights share SBUF slots with the attention exp-buffers (tag "e"):
they are dead once the projections finish, exactly when the exp buffers
start rotating.  Attention runs a depth-2 software pipeline (st/exp two
chunks ahead of pv) so the scalar engine's exp throughput (~8.7us/chunk vs
~7.0us of tensor work) never stalls the PE.  Startup DMAs are sliced per
4-ht block, interleaving wq and x so the first q-matmul starts after ~1MB
of traffic instead of 4MB.
"""

import numpy as np
import ml_dtypes

B, S, H = 2, 2048, 2048
NH, HD = 16, 128
P = 128
HT = H // P            # 16 hidden-dim tiles
G = 4                  # heads per core
GH = G * HD            # 512: head-group width per core
SCALE = 1.0 / float(np.sqrt(HD))
WS = 128.0             # fp8 q/k weight pre-scale
N_CORES = 8
XC = 512               # phase-1 x streaming chunk (s elements)
QC = 512               # attention query chunk
KT = S // P            # 16 key tiles

_CACHE = {}


def _build():
    import concourse.mybir as mybir
    import concourse.tile as tile
    from concourse import bacc

    dt = mybir.dt
    Alu = mybir.AluOpType
    Act = mybir.ActivationFunctionType

    nc = bacc.Bacc("TRN2", target_bir_lowering=False, debug=False,
                   enable_asserts=False)

    xt_d = nc.dram_tensor("xt", [H, S], dt.bfloat16, kind="ExternalInput").ap()
    xt8_d = nc.dram_tensor("xt8", [H, S], dt.float8e4,
                           kind="ExternalInput").ap()
    # q/k weights are fp8 (pre-scaled x128 on host), head-major so each
    # head's block is contiguous in DRAM.
    wqt_d = nc.dram_tensor("wqt8", [G * H, HD], dt.float8e4,
                           kind="ExternalInput").ap()
    wkt_d = nc.dram_tensor("wkt8", [G * H, HD], dt.float8e4,
                           kind="ExternalInput").ap()
    wvt_d = nc.dram_tensor("wvt", [H, GH], dt.bfloat16, kind="ExternalInput").ap()
    bqs_d = nc.dram_tensor("bqs", [P, G], dt.float32, kind="ExternalInput").ap()
    bk_d = nc.dram_tensor("bk", [P, G], dt.float32, kind="ExternalInput").ap()
    wot_d = nc.dram_tensor("wot", [GH, H], dt.bfloat16, kind="ExternalInput").ap()
    out_d = nc.dram_tensor("partial", [S, H], dt.bfloat16,
                           kind="ExternalOutput").ap()

    xt_r = xt_d.rearrange("(ht p) s -> p ht s", p=P)      # [128, 16, 2048]
    xt8_r = xt8_d.rearrange("(ht p) s -> p ht s", p=P)
    wqt_r = wqt_d.rearrange("(g ht p) d -> p g ht d", g=G, p=P)  # [128,4,16,128]
    wkt_r = wkt_d.rearrange("(g ht p) d -> p g ht d", g=G, p=P)
    wvt_r = wvt_d.rearrange("(ht p) o -> p ht o", p=P)
    wot_r = wot_d.rearrange("(g p) o -> p g o", p=P)      # [128, 4, 2048]

    NXC = S // XC      # 4
    NQC = S // QC      # 4

    with tile.TileContext(nc) as tc:
        with (
            tc.tile_pool(name="consts", bufs=1) as consts,
            tc.tile_pool(name="wpool", bufs=1) as wpool,
            tc.tile_pool(name="xpool", bufs=2) as xpool,
            tc.tile_pool(name="big", bufs=1) as big,
            tc.tile_pool(name="epool", bufs=4) as epool,
            tc.tile_pool(name="small", bufs=2) as small,
            tc.tile_pool(name="psum", bufs=2, space="PSUM") as psum,
        ):
            # --- startup DMAs, critical-path first.  wq slices issue from
            # the Sync queue while the x slices issue in parallel from the
            # (otherwise idle) GpSimd queue: DMA issue instructions cost
            # ~0.7-2us each, so serializing them on one engine delays the
            # first matmul.  First pieces are 2-ht (256KB) so the first
            # q-matmul group (which consumes ht ascending) starts early. ---
            # packed fp8 q+k weights: cols 0:GH = q heads, GH:2GH = k heads
            w8_sb = epool.tile([P, HT, 2 * GH], dt.float8e4, tag="e",
                               name="w8_sb")
            xt80_sb = xpool.tile([P, HT, XC], dt.float8e4, tag="xt8",
                                 name="xt80_sb")
            for h in range(G):
                nc.sync.dma_start(w8_sb[:, :, h * HD:(h + 1) * HD],
                                  wqt_r[:, h, :, :])
            for lo, hi in ((0, 8), (8, 16)):
                sl = slice(lo, hi)
                nc.gpsimd.dma_start(xt80_sb[:, sl, :], xt8_r[:, sl, 0:XC])
            for h in range(G):
                nc.sync.dma_start(w8_sb[:, :, GH + h * HD:GH + (h + 1) * HD],
                                  wkt_r[:, h, :, :])
            bqs_sb = consts.tile([P, G], dt.float32)
            nc.gpsimd.dma_start(bqs_sb[:], bqs_d)
            bk_sb = consts.tile([P, G], dt.float32)
            nc.gpsimd.dma_start(bk_sb[:], bk_d)
            ones_sb = consts.tile([P, P], dt.bfloat16)
            nc.vector.memset(ones_sb[:], 1.0)
            # PE warmup: TRN2 ramps the PE clock (0.65 -> 2.4 GHz) only while
            # the array is busy; throwaway matmuls during the startup DMA
            # wait mean the first real matmuls run at speed.
            for _ in range(16):
                wps = psum.tile([P, 512], dt.float32, tag="mm")
                nc.tensor.matmul(wps[:, 0:P], ones_sb[:], ones_sb[:],
                                 start=True, stop=True)
            # x (bf16) for the v projection; v stays bf16 for accuracy
            xt0_sb = xpool.tile([P, HT, XC], dt.bfloat16, tag="xt",
                                name="xt0_sb")
            nc.sync.dma_start(xt0_sb[:], xt_r[:, :, 0:XC])
            wv_sb = epool.tile([P, HT, GH], dt.bfloat16, tag="e", name="wv_sb")
            nc.sync.dma_start(wv_sb[:, 0:8, :], wvt_r[:, 0:8, :])
            nc.sync.dma_start(wv_sb[:, 8:16, :], wvt_r[:, 8:16, :])

            qt_sb = big.tile([P, G, S], dt.bfloat16)   # q^T, scale+bias applied
            kt_sb = big.tile([P, G, S], dt.bfloat16)   # k^T, bias applied
            v_sb = big.tile([P, KT, GH], dt.bfloat16)  # v natural [s, o]
            at_tiles = {}                              # attn out^T, ring per qc

            # ---------------- Phase 1: QKV projections ----------------
            DR = mybir.MatmulPerfMode.DoubleRow
            for xc in range(NXC):
                if xc == 0:
                    xt8_sb = xt80_sb
                    xt_sb = xt0_sb
                else:
                    xt8_sb = xpool.tile([P, HT, XC], dt.float8e4, tag="xt8",
                                        name="xt8_sb")
                    nc.gpsimd.dma_start(xt8_sb[:],
                                        xt8_r[:, :, xc * XC:(xc + 1) * XC])
                    xt_sb = xpool.tile([P, HT, XC], dt.bfloat16, tag="xt",
                                       name="xt_sb")
                    nc.gpsimd.dma_start(xt_sb[:], xt_r[:, :, xc * XC:(xc + 1) * XC])
                sl = slice(xc * XC, (xc + 1) * XC)
                for h in range(G):
                    psq = psum.tile([P, 512], dt.float32, tag="mm")
                    for hp in range(HT // 2):
                        nc.tensor.matmul(psq,
                                         w8_sb[:, 2 * hp:2 * hp + 2,
                                               h * HD:(h + 1) * HD],
                                         xt8_sb[:, 2 * hp:2 * hp + 2, :],
                                         start=(hp == 0), stop=(hp == HT // 2 - 1),
                                         perf_mode=DR)
                    nc.vector.tensor_scalar(qt_sb[:, h, sl], psq,
                                            SCALE / WS, bqs_sb[:, h:h + 1],
                                            Alu.mult, Alu.add)
                for h in range(G):
                    psk = psum.tile([P, 512], dt.float32, tag="mm")
                    for hp in range(HT // 2):
                        nc.tensor.matmul(psk,
                                         w8_sb[:, 2 * hp:2 * hp + 2,
                                               GH + h * HD:GH + (h + 1) * HD],
                                         xt8_sb[:, 2 * hp:2 * hp + 2, :],
                                         start=(hp == 0), stop=(hp == HT // 2 - 1),
                                         perf_mode=DR)
                    nc.vector.tensor_scalar(kt_sb[:, h, sl], psk,
                                            1.0 / WS, bk_sb[:, h:h + 1],
                                            Alu.mult, Alu.add)
                for sv in range(XC // P):
                    sm = xc * (XC // P) + sv
                    psv = psum.tile([P, 512], dt.float32, tag="mm")
                    for ht in range(HT):
                        nc.tensor.matmul(psv,
                                         xt_sb[:, ht, sv * P:(sv + 1) * P],
                                         wv_sb[:, ht, :],
                                         start=(ht == 0), stop=(ht == HT - 1))
                    nc.vector.tensor_copy(out=v_sb[:, sm, :], in_=psv)

            # out-proj weights: needed only from the first proj (~mid-kernel)
            wo_sb = wpool.tile([P, G, H], dt.bfloat16)
            nc.sync.dma_start(wo_sb[:], wot_r)

            # -------- Phase 2+3: attention + out-proj (sw-pipelined) --------
            def emit_st_exp(h, qc, pvjob=None):
                # ST^T = k^T.T @ q^T per 128-key tile; exp on ACT in 2-bank
                # batches (halves the per-ACTIVATE overhead).  The previous
                # chunk's pv matmuls (whose exp inputs are long done) are
                # interleaved between st pairs so the tensor engine never
                # stalls on the ACT-paced st psum ring.
                e_sb = epool.tile([P, KT, QC], dt.bfloat16, tag="e",
                                  name="e_sb")
                for km in range(0, KT, 2):
                    ps = psum.tile([P, 2, QC], dt.float32, tag="st")
                    for j in range(2):
                        nc.tensor.matmul(ps[:, j, :],
                                         kt_sb[:, h, (km + j) * P:(km + j + 1) * P],
                                         qt_sb[:, h, qc * QC:(qc + 1) * QC],
                                         start=True, stop=True)
                    if pvjob is not None:
                        pv, ph, pe = pvjob
                        for j in range(2):
                            nc.tensor.matmul(pv,
                                             v_sb[:, km + j, ph * HD:(ph + 1) * HD],
                                             pe[:, km + j, :],
                                             start=(km + j == 0),
                                             stop=(km + j == KT - 1))
                    nc.scalar.activation(e_sb[:, km:km + 2, :], ps, Act.Exp)
                return e_sb

            def alloc_pv(h, qc):
                if h == 0:
                    at_tiles[qc] = small.tile([P, G, QC], dt.bfloat16,
                                              tag="at", bufs=2, name="at_t")
                pv = psum.tile([P, QC], dt.float32, tag="pv", bufs=2)
                return pv

            def emit_pv_plain(pv, h, e_sb):
                for km in range(KT):
                    nc.tensor.matmul(pv, v_sb[:, km, h * HD:(h + 1) * HD],
                                     e_sb[:, km, :],
                                     start=(km == 0), stop=(km == KT - 1))

            def emit_z_norm(pv, h, qc, e_sb):
                # z: DVE add-ladder over the 16 key-tiles of e (bf16, packed
                # SBUF operands -> fast DVE mode), then ONE ones-matmul to
                # partition-reduce, landing z broadcast across partitions.
                lA = small.tile([P, 4, QC], dt.bfloat16, tag="l4")
                nc.vector.tensor_add(out=lA[:], in0=e_sb[:, 0:4, :],
                                     in1=e_sb[:, 4:8, :])
                lB = small.tile([P, 4, QC], dt.bfloat16, tag="l4")
                nc.vector.tensor_add(out=lB[:], in0=e_sb[:, 8:12, :],
                                     in1=e_sb[:, 12:16, :])
                lC = small.tile([P, 2, QC], dt.bfloat16, tag="l2")
                nc.vector.tensor_add(out=lC[:], in0=lA[:, 0:2, :],
                                     in1=lA[:, 2:4, :])
                lD = small.tile([P, 2, QC], dt.bfloat16, tag="l2")
                nc.vector.tensor_add(out=lD[:], in0=lB[:, 0:2, :],
                                     in1=lB[:, 2:4, :])
                lE = small.tile([P, 2, QC], dt.bfloat16, tag="le", bufs=1)
                nc.vector.tensor_add(out=lE[:], in0=lC[:], in1=lD[:])
                esum = small.tile([P, QC], dt.bfloat16, tag="es", bufs=1)
                nc.vector.tensor_add(out=esum[:], in0=lE[:, 0, :],
                                     in1=lE[:, 1, :])
                # z shares the "mm" psum ring (its own bank would push the
                # total over 8 now that pv is double-buffered)
                z = psum.tile([P, QC], dt.float32, tag="mm")
                nc.tensor.matmul(z, ones_sb[:], esum[:], start=True, stop=True)
                zi = small.tile([P, QC], dt.float32, tag="zi", bufs=1)
                nc.vector.reciprocal_approx_fast(out=zi[:], in_=z)
                nc.vector.tensor_mul(out=at_tiles[qc][:, h, :],
                                     in0=pv, in1=zi[:])

            def emit_proj(qc, last=False):
                for sv in range(QC // P):
                    sm = qc * (QC // P) + sv
                    ob = None
                    for oc in range(H // 512):
                        pp = psum.tile([P, 512], dt.float32, tag="mm")
                        for g in range(G):
                            nc.tensor.matmul(pp,
                                             at_tiles[qc][:, g, sv * P:(sv + 1) * P],
                                             wo_sb[:, g, oc * 512:(oc + 1) * 512],
                                             start=(g == 0), stop=(g == G - 1))
                        if oc % 2 == 0:
                            ob = small.tile([P, 2, 512], dt.bfloat16, tag="ob",
                                            bufs=2)
                        # in the final group, split the drain copies across
                        # DVE and ACT so the tail isn't serialized on one
                        # engine (Copy is in every ACT table set: no reload;
                        # GpSimd cannot read PSUM on TRN2).
                        if last and oc % 2 == 1:
                            nc.scalar.copy(ob[:, oc % 2, :], pp)
                        else:
                            nc.vector.tensor_copy(out=ob[:, oc % 2, :], in_=pp)
                        if oc % 2 == 1:
                            nc.sync.dma_start(
                                out_d[sm * P:(sm + 1) * P,
                                      (oc - 1) * 512:(oc + 1) * 512],
                                ob[:])

            chunks = [(h, qc) for qc in range(NQC) for h in range(G)]
            emitted = []
            for i, (h, qc) in enumerate(chunks):
                if i >= 2:
                    ph, pqc, pe = emitted[i - 2]
                    pv = alloc_pv(ph, pqc)
                    e = emit_st_exp(h, qc, pvjob=(pv, ph, pe))
                    emitted.append((h, qc, e))
                    emit_z_norm(pv, ph, pqc, pe)
                    if ph == G - 1:
                        emit_proj(pqc)
                else:
                    e = emit_st_exp(h, qc)
                    emitted.append((h, qc, e))
            for i in (len(chunks) - 2, len(chunks) - 1):
                ph, pqc, pe = emitted[i]
                pv = alloc_pv(ph, pqc)
                emit_pv_plain(pv, ph, pe)
                emit_z_norm(pv, ph, pqc, pe)
                if ph == G - 1:
                    emit_proj(pqc, last=(i == len(chunks) - 1))

    nc.compile()
    return nc


def _get_nc():
    if "nc" not in _CACHE:
        _CACHE["nc"] = _build()
    return _CACHE["nc"]


def _head_major(wt):
    # [H, GH] -> head-major [G*H, HD]
    return np.ascontiguousarray(
        wt.reshape(H, G, HD).transpose(1, 0, 2).reshape(G * H, HD))


def _make_in_maps(x, w_qkv, b_qkv, w_out):
    bf = ml_dtypes.bfloat16
    f8 = ml_dtypes.float8_e4m3
    f32 = np.float32
    in_maps = []
    for c in range(N_CORES):
        b = c // 4
        g = c % 4
        lo = GH * g
        hi = GH * (g + 1)
        xt = np.ascontiguousarray(x[b].T)
        xt8 = xt.astype(f8)
        wqt8 = _head_major(w_qkv[lo:hi, :].T * WS).astype(f8)
        wkt8 = _head_major(w_qkv[H + lo:H + hi, :].T * WS).astype(f8)
        wvt = np.ascontiguousarray(w_qkv[2 * H + lo:2 * H + hi, :].T).astype(bf)
        bqs = np.ascontiguousarray(
            (b_qkv[lo:hi] * SCALE).astype(f32).reshape(G, P).T)
        bk = np.ascontiguousarray(
            b_qkv[H + lo:H + hi].astype(f32).reshape(G, P).T)
        wot = np.ascontiguousarray(w_out[:, lo:hi].T).astype(bf)
        in_maps.append({"xt": xt.astype(bf), "xt8": xt8, "wqt8": wqt8,
                        "wkt8": wkt8, "wvt": wvt,
                        "bqs": bqs, "bk": bk, "wot": wot})
    return in_maps


def kernel(x, w_qkv, b_qkv, w_out, b_out):
    import os
    import sys

    x = np.asarray(x, dtype=np.float32)
    w_qkv = np.asarray(w_qkv, dtype=np.float32)
    b_qkv = np.asarray(b_qkv, dtype=np.float32)
    w_out = np.asarray(w_out, dtype=np.float32)
    b_out = np.asarray(b_out, dtype=np.float32)

    from concourse.bass_utils import run_bass_kernel_spmd

    # NTFF tracing under axon needs the antenv.axon_hooks shim (test.py
    # installs it); without it a stray BASS_TRACE=1 in the environment would
    # crash the run — disable tracing in that case.
    if "antenv.axon_hooks" not in sys.modules:
        os.environ["BASS_NEVER_TRACE"] = "1"

    nc = _get_nc()
    in_maps = _make_in_maps(x, w_qkv, b_qkv, w_out)
    res = run_bass_kernel_spmd(nc, in_maps, core_ids=list(range(N_CORES)))
    _CACHE["last_results"] = res
    partials = [r["partial"] for r in res.results]

    bv = b_qkv[2 * H:3 * H]
    bias = b_out + w_out @ bv          # folded v-bias contribution
    out = np.empty((B, S, H), np.float32)
    for b in range(B):
        acc = partials[4 * b].astype(np.float32)
        for g in range(1, 4):
            acc += partials[4 * b + g].astype(np.float32)
        out[b] = acc + bias
    return out


# revision 14
# speedup vs baseline: 1.2089x; 1.2089x over previous
"""Trainium2 Bass kernel for NoTPAttention — fp8 q/k projection variant.

(dense transformer block:
fused QKV projection -> multi-head attention -> output projection).

Sharding (8 NeuronCores): core c handles batch b = c // 4 and the 4 heads
g = 4*(c % 4) .. 4*(c % 4)+3 (head-parallel tensor parallelism).  Each core
computes its heads' partial out-projection [S, H] in bf16; the host sums the
4 partials per batch in fp32 and adds the (folded) biases.

Numerics: all matmuls run in bf16 with fp32 PSUM accumulation.  Softmax is
computed without max-subtraction (scores are bounded, |s| < ~3.5) with the
normalization deferred to the attention *output*:
    attnT[d, q] = (sum_k v[k, d] * exp(sT[k, q])) / z[q],  z = sum_k exp
The denominator is NOT a full ones-matmul over every e-tile (that would cost
as much tensor time as the pv matmul).  Instead the DVE pre-reduces the 16
key-tiles of e elementwise (a 6-instruction bf16 add-ladder, cheap on the
otherwise idle vector engine), and a SINGLE ones-matmul per chunk
partition-reduces the [128, QC] ladder result, landing z already broadcast
across partitions.  This cuts the z tensor cost 16x (~51us/core).
The v-bias is dropped in-kernel: after normalization it contributes exactly
b_v to every row, so the host folds w_out @ b_v into the output bias.

Layout notes: qT/kT/attnT live as [128 (head-dim), head, seq] so every
matmul contracts over a full 128-partition tile with no transposes anywhere.
The qkv weights share SBUF slots with the attention exp-buffers (tag "e"):
they are dead once the projections finish, exactly when the exp buffers
start rotating.  Attention runs a depth-2 software pipeline (st/exp two
chunks ahead of pv) so the scalar engine's exp throughput (~8.7us/chunk vs
~7.0us of tensor work) never stalls the PE.  Startup DMAs are sliced per
4-ht block, interleaving wq and x so the first q-matmul starts after ~1MB
of traffic instead of 4MB.
"""

import numpy as np
import ml_dtypes

B, S, H = 2, 2048, 2048
NH, HD = 16, 128
P = 128
HT = H // P            # 16 hidden-dim tiles
G = 4                  # heads per core
GH = G * HD            # 512: head-group width per core
SCALE = 1.0 / float(np.sqrt(HD))
WS = 128.0             # fp8 q/k weight pre-scale
N_CORES = 8
XC = 512               # phase-1 x streaming chunk (s elements)
QC = 512               # attention query chunk
KT = S // P            # 16 key tiles

_CACHE = {}


def _build():
    import concourse.mybir as mybir
    import concourse.tile as tile
    from concourse import bacc

    dt = mybir.dt
    Alu = mybir.AluOpType
    Act = mybir.ActivationFunctionType

    nc = bacc.Bacc("TRN2", target_bir_lowering=False, debug=False,
                   enable_asserts=False)

    xt_d = nc.dram_tensor("xt", [H, S], dt.bfloat16, kind="ExternalInput").ap()
    xt8_d = nc.dram_tensor("xt8", [H, S], dt.float8e4,
                           kind="ExternalInput").ap()
    # q/k weights are fp8 (pre-scaled x128 on host), head-major so each
    # head's block is contiguous in DRAM.
    wqt_d = nc.dram_tensor("wqt8", [G * H, HD], dt.float8e4,
                           kind="ExternalInput").ap()
    wkt_d = nc.dram_tensor("wkt8", [G * H, HD], dt.float8e4,
                           kind="ExternalInput").ap()
    wvt_d = nc.dram_tensor("wvt", [H, GH], dt.bfloat16, kind="ExternalInput").ap()
    bqs_d = nc.dram_tensor("bqs", [P, G], dt.float32, kind="ExternalInput").ap()
    bk_d = nc.dram_tensor("bk", [P, G], dt.float32, kind="ExternalInput").ap()
    wot_d = nc.dram_tensor("wot", [GH, H], dt.bfloat16, kind="ExternalInput").ap()
    out_d = nc.dram_tensor("partial", [S, H], dt.bfloat16,
                           kind="ExternalOutput").ap()

    xt_r = xt_d.rearrange("(ht p) s -> p ht s", p=P)      # [128, 16, 2048]
    xt8_r = xt8_d.rearrange("(ht p) s -> p ht s", p=P)
    wqt_r = wqt_d.rearrange("(g ht p) d -> p g ht d", g=G, p=P)  # [128,4,16,128]
    wkt_r = wkt_d.rearrange("(g ht p) d -> p g ht d", g=G, p=P)
    wvt_r = wvt_d.rearrange("(ht p) o -> p ht o", p=P)
    wot_r = wot_d.rearrange("(g p) o -> p g o", p=P)      # [128, 4, 2048]

    NXC = S // XC      # 4
    NQC = S // QC      # 4

    with tile.TileContext(nc) as tc:
        with (
            tc.tile_pool(name="consts", bufs=1) as consts,
            tc.tile_pool(name="wpool", bufs=1) as wpool,
            tc.tile_pool(name="xpool", bufs=2) as xpool,
            tc.tile_pool(name="big", bufs=1) as big,
            tc.tile_pool(name="epool", bufs=4) as epool,
            tc.tile_pool(name="small", bufs=2) as small,
            tc.tile_pool(name="psum", bufs=2, space="PSUM") as psum,
        ):
            # --- startup DMAs, critical-path first.  wq slices issue from
            # the Sync queue while the x slices issue in parallel from the
            # (otherwise idle) GpSimd queue: DMA issue instructions cost
            # ~0.7-2us each, so serializing them on one engine delays the
            # first matmul.  First pieces are 2-ht (256KB) so the first
            # q-matmul group (which consumes ht ascending) starts early. ---
            # packed fp8 q+k weights: cols 0:GH = q heads, GH:2GH = k heads
            w8_sb = epool.tile([P, HT, 2 * GH], dt.float8e4, tag="e",
                               name="w8_sb")
            xt80_sb = xpool.tile([P, HT, XC], dt.float8e4, tag="xt8",
                                 name="xt80_sb")
            for h in range(G):
                nc.sync.dma_start(w8_sb[:, :, h * HD:(h + 1) * HD],
                                  wqt_r[:, h, :, :])
            for lo, hi in ((0, 8), (8, 16)):
                sl = slice(lo, hi)
                nc.gpsimd.dma_start(xt80_sb[:, sl, :], xt8_r[:, sl, 0:XC])
            # x (bf16) for the v projection; v stays bf16 for accuracy.
            # Ordering matters: k weights and the v-pass inputs (xt0, wv)
            # must land before their first consumers (~7us and ~14us after
            # the q groups start), so they interleave here.
            xt0_sb = xpool.tile([P, HT, XC], dt.bfloat16, tag="xt",
                                name="xt0_sb")
            nc.sync.dma_start(xt0_sb[:, 0:8, :], xt_r[:, 0:8, 0:XC])
            for h in range(G):
                nc.sync.dma_start(w8_sb[:, :, GH + h * HD:GH + (h + 1) * HD],
                                  wkt_r[:, h, :, :])
            nc.sync.dma_start(xt0_sb[:, 8:16, :], xt_r[:, 8:16, 0:XC])
            wv_sb = epool.tile([P, HT, GH], dt.bfloat16, tag="e", name="wv_sb")
            nc.sync.dma_start(wv_sb[:, 0:8, :], wvt_r[:, 0:8, :])
            nc.sync.dma_start(wv_sb[:, 8:16, :], wvt_r[:, 8:16, :])
            bqs_sb = consts.tile([P, G], dt.float32)
            nc.gpsimd.dma_start(bqs_sb[:], bqs_d)
            bk_sb = consts.tile([P, G], dt.float32)
            nc.gpsimd.dma_start(bk_sb[:], bk_d)
            ones_sb = consts.tile([P, P], dt.bfloat16)
            nc.vector.memset(ones_sb[:], 1.0)
            # PE warmup: TRN2 ramps the PE clock (0.65 -> 2.4 GHz) only while
            # the array is busy; throwaway matmuls during the startup DMA
            # wait mean the first real matmuls run at speed.
            for _ in range(16):
                wps = psum.tile([P, 512], dt.float32, tag="mm")
                nc.tensor.matmul(wps[:, 0:P], ones_sb[:], ones_sb[:],
                                 start=True, stop=True)

            qt_sb = big.tile([P, G, S], dt.bfloat16)   # q^T, scale+bias applied
            kt_sb = big.tile([P, G, S], dt.bfloat16)   # k^T, bias applied
            v_sb = big.tile([P, KT, GH], dt.bfloat16)  # v natural [s, o]
            at_tiles = {}                              # attn out^T, ring per qc

            # ---------------- Phase 1: QKV projections ----------------
            DR = mybir.MatmulPerfMode.DoubleRow
            for xc in range(NXC):
                if xc == 0:
                    xt8_sb = xt80_sb
                    xt_sb = xt0_sb
                else:
                    xt8_sb = xpool.tile([P, HT, XC], dt.float8e4, tag="xt8",
                                        name="xt8_sb")
                    nc.gpsimd.dma_start(xt8_sb[:],
                                        xt8_r[:, :, xc * XC:(xc + 1) * XC])
                    xt_sb = xpool.tile([P, HT, XC], dt.bfloat16, tag="xt",
                                       name="xt_sb")
                    nc.sync.dma_start(xt_sb[:], xt_r[:, :, xc * XC:(xc + 1) * XC])
                sl = slice(xc * XC, (xc + 1) * XC)
                for h in range(G):
                    psq = psum.tile([P, 512], dt.float32, tag="mm")
                    for hp in range(HT // 2):
                        nc.tensor.matmul(psq,
                                         w8_sb[:, 2 * hp:2 * hp + 2,
                                               h * HD:(h + 1) * HD],
                                         xt8_sb[:, 2 * hp:2 * hp + 2, :],
                                         start=(hp == 0), stop=(hp == HT // 2 - 1),
                                         perf_mode=DR)
                    nc.vector.tensor_scalar(qt_sb[:, h, sl], psq,
                                            SCALE / WS, bqs_sb[:, h:h + 1],
                                            Alu.mult, Alu.add)
                for h in range(G):
                    psk = psum.tile([P, 512], dt.float32, tag="mm")
                    for hp in range(HT // 2):
                        nc.tensor.matmul(psk,
                                         w8_sb[:, 2 * hp:2 * hp + 2,
                                               GH + h * HD:GH + (h + 1) * HD],
                                         xt8_sb[:, 2 * hp:2 * hp + 2, :],
                                         start=(hp == 0), stop=(hp == HT // 2 - 1),
                                         perf_mode=DR)
                    nc.vector.tensor_scalar(kt_sb[:, h, sl], psk,
                                            1.0 / WS, bk_sb[:, h:h + 1],
                                            Alu.mult, Alu.add)
                for sv in range(XC // P):
                    sm = xc * (XC // P) + sv
                    psv = psum.tile([P, 512], dt.float32, tag="mm")
                    for ht in range(HT):
                        nc.tensor.matmul(psv,
                                         xt_sb[:, ht, sv * P:(sv + 1) * P],
                                         wv_sb[:, ht, :],
                                         start=(ht == 0), stop=(ht == HT - 1))
                    nc.vector.tensor_copy(out=v_sb[:, sm, :], in_=psv)

            # out-proj weights: needed only from the first proj (~mid-kernel)
            wo_sb = wpool.tile([P, G, H], dt.bfloat16)
            nc.sync.dma_start(wo_sb[:], wot_r)

            # -------- Phase 2+3: attention + out-proj (sw-pipelined) --------
            def emit_st_exp(h, qc, pvjob=None):
                # ST^T = k^T.T @ q^T per 128-key tile; exp on ACT in 2-bank
                # batches (halves the per-ACTIVATE overhead).  The previous
                # chunk's pv matmuls (whose exp inputs are long done) are
                # interleaved between st pairs so the tensor engine never
                # stalls on the ACT-paced st psum ring.
                e_sb = epool.tile([P, KT, QC], dt.bfloat16, tag="e",
                                  name="e_sb")
                for km in range(0, KT, 2):
                    ps = psum.tile([P, 2, QC], dt.float32, tag="st")
                    for j in range(2):
                        nc.tensor.matmul(ps[:, j, :],
                                         kt_sb[:, h, (km + j) * P:(km + j + 1) * P],
                                         qt_sb[:, h, qc * QC:(qc + 1) * QC],
                                         start=True, stop=True)
                    if pvjob is not None:
                        pv, ph, pe = pvjob
                        for j in range(2):
                            nc.tensor.matmul(pv,
                                             v_sb[:, km + j, ph * HD:(ph + 1) * HD],
                                             pe[:, km + j, :],
                                             start=(km + j == 0),
                                             stop=(km + j == KT - 1))
                    nc.scalar.activation(e_sb[:, km:km + 2, :], ps, Act.Exp)
                return e_sb

            def alloc_pv(h, qc):
                if h == 0:
                    at_tiles[qc] = small.tile([P, G, QC], dt.bfloat16,
                                              tag="at", bufs=2, name="at_t")
                pv = psum.tile([P, QC], dt.float32, tag="pv", bufs=2)
                return pv

            def emit_pv_plain(pv, h, e_sb):
                for km in range(KT):
                    nc.tensor.matmul(pv, v_sb[:, km, h * HD:(h + 1) * HD],
                                     e_sb[:, km, :],
                                     start=(km == 0), stop=(km == KT - 1))

            def emit_z_norm(pv, h, qc, e_sb):
                # z: DVE add-ladder over the 16 key-tiles of e (bf16, packed
                # SBUF operands -> fast DVE mode), then ONE ones-matmul to
                # partition-reduce, landing z broadcast across partitions.
                lA = small.tile([P, 4, QC], dt.bfloat16, tag="l4")
                nc.vector.tensor_add(out=lA[:], in0=e_sb[:, 0:4, :],
                                     in1=e_sb[:, 4:8, :])
                lB = small.tile([P, 4, QC], dt.bfloat16, tag="l4")
                nc.vector.tensor_add(out=lB[:], in0=e_sb[:, 8:12, :],
                                     in1=e_sb[:, 12:16, :])
                lC = small.tile([P, 2, QC], dt.bfloat16, tag="l2")
                nc.vector.tensor_add(out=lC[:], in0=lA[:, 0:2, :],
                                     in1=lA[:, 2:4, :])
                lD = small.tile([P, 2, QC], dt.bfloat16, tag="l2")
                nc.vector.tensor_add(out=lD[:], in0=lB[:, 0:2, :],
                                     in1=lB[:, 2:4, :])
                lE = small.tile([P, 2, QC], dt.bfloat16, tag="le", bufs=1)
                nc.vector.tensor_add(out=lE[:], in0=lC[:], in1=lD[:])
                esum = small.tile([P, QC], dt.bfloat16, tag="es", bufs=1)
                nc.vector.tensor_add(out=esum[:], in0=lE[:, 0, :],
                                     in1=lE[:, 1, :])
                # z shares the "mm" psum ring (its own bank would push the
                # total over 8 now that pv is double-buffered)
                z = psum.tile([P, QC], dt.float32, tag="mm")
                nc.tensor.matmul(z, ones_sb[:], esum[:], start=True, stop=True)
                zi = small.tile([P, QC], dt.float32, tag="zi", bufs=1)
                nc.vector.reciprocal_approx_fast(out=zi[:], in_=z)
                nc.vector.tensor_mul(out=at_tiles[qc][:, h, :],
                                     in0=pv, in1=zi[:])

            def emit_proj(qc, last=False):
                for sv in range(QC // P):
                    sm = qc * (QC // P) + sv
                    ob = None
                    for oc in range(H // 512):
                        pp = psum.tile([P, 512], dt.float32, tag="mm")
                        for g in range(G):
                            nc.tensor.matmul(pp,
                                             at_tiles[qc][:, g, sv * P:(sv + 1) * P],
                                             wo_sb[:, g, oc * 512:(oc + 1) * 512],
                                             start=(g == 0), stop=(g == G - 1))
                        if oc % 2 == 0:
                            ob = small.tile([P, 2, 512], dt.bfloat16, tag="ob",
                                            bufs=2)
                        # in the final group, split the drain copies across
                        # DVE and ACT so the tail isn't serialized on one
                        # engine (Copy is in every ACT table set: no reload;
                        # GpSimd cannot read PSUM on TRN2).
                        if last and oc % 2 == 1:
                            nc.scalar.copy(ob[:, oc % 2, :], pp)
                        else:
                            nc.vector.tensor_copy(out=ob[:, oc % 2, :], in_=pp)
                        if oc % 2 == 1:
                            nc.sync.dma_start(
                                out_d[sm * P:(sm + 1) * P,
                                      (oc - 1) * 512:(oc + 1) * 512],
                                ob[:])

            chunks = [(h, qc) for qc in range(NQC) for h in range(G)]
            emitted = []
            for i, (h, qc) in enumerate(chunks):
                if i >= 2:
                    ph, pqc, pe = emitted[i - 2]
                    pv = alloc_pv(ph, pqc)
                    e = emit_st_exp(h, qc, pvjob=(pv, ph, pe))
                    emitted.append((h, qc, e))
                    emit_z_norm(pv, ph, pqc, pe)
                    if ph == G - 1:
                        emit_proj(pqc)
                else:
                    e = emit_st_exp(h, qc)
                    emitted.append((h, qc, e))
            for i in (len(chunks) - 2, len(chunks) - 1):
                ph, pqc, pe = emitted[i]
                pv = alloc_pv(ph, pqc)
                emit_pv_plain(pv, ph, pe)
                emit_z_norm(pv, ph, pqc, pe)
                if ph == G - 1:
                    emit_proj(pqc, last=(i == len(chunks) - 1))

    nc.compile()
    return nc


def _get_nc():
    if "nc" not in _CACHE:
        _CACHE["nc"] = _build()
    return _CACHE["nc"]


def _head_major(wt):
    # [H, GH] -> head-major [G*H, HD]
    return np.ascontiguousarray(
        wt.reshape(H, G, HD).transpose(1, 0, 2).reshape(G * H, HD))


def _make_in_maps(x, w_qkv, b_qkv, w_out):
    bf = ml_dtypes.bfloat16
    f8 = ml_dtypes.float8_e4m3
    f32 = np.float32
    in_maps = []
    for c in range(N_CORES):
        b = c // 4
        g = c % 4
        lo = GH * g
        hi = GH * (g + 1)
        xt = np.ascontiguousarray(x[b].T)
        xt8 = xt.astype(f8)
        wqt8 = _head_major(w_qkv[lo:hi, :].T * WS).astype(f8)
        wkt8 = _head_major(w_qkv[H + lo:H + hi, :].T * WS).astype(f8)
        wvt = np.ascontiguousarray(w_qkv[2 * H + lo:2 * H + hi, :].T).astype(bf)
        bqs = np.ascontiguousarray(
            (b_qkv[lo:hi] * SCALE).astype(f32).reshape(G, P).T)
        bk = np.ascontiguousarray(
            b_qkv[H + lo:H + hi].astype(f32).reshape(G, P).T)
        wot = np.ascontiguousarray(w_out[:, lo:hi].T).astype(bf)
        in_maps.append({"xt": xt.astype(bf), "xt8": xt8, "wqt8": wqt8,
                        "wkt8": wkt8, "wvt": wvt,
                        "bqs": bqs, "bk": bk, "wot": wot})
    return in_maps


def kernel(x, w_qkv, b_qkv, w_out, b_out):
    import os
    import sys

    x = np.asarray(x, dtype=np.float32)
    w_qkv = np.asarray(w_qkv, dtype=np.float32)
    b_qkv = np.asarray(b_qkv, dtype=np.float32)
    w_out = np.asarray(w_out, dtype=np.float32)
    b_out = np.asarray(b_out, dtype=np.float32)

    from concourse.bass_utils import run_bass_kernel_spmd

    # NTFF tracing under axon needs the antenv.axon_hooks shim (test.py
    # installs it); without it a stray BASS_TRACE=1 in the environment would
    # crash the run — disable tracing in that case.
    if "antenv.axon_hooks" not in sys.modules:
        os.environ["BASS_NEVER_TRACE"] = "1"

    nc = _get_nc()
    in_maps = _make_in_maps(x, w_qkv, b_qkv, w_out)
    res = run_bass_kernel_spmd(nc, in_maps, core_ids=list(range(N_CORES)))
    _CACHE["last_results"] = res
    partials = [r["partial"] for r in res.results]

    bv = b_qkv[2 * H:3 * H]
    bias = b_out + w_out @ bv          # folded v-bias contribution
    out = np.empty((B, S, H), np.float32)
    for b in range(B):
        acc = partials[4 * b].astype(np.float32)
        for g in range(1, 4):
            acc += partials[4 * b + g].astype(np.float32)
        out[b] = acc + bias
    return out


# revision 15
# speedup vs baseline: 1.2351x; 1.0217x over previous
"""Trainium2 Bass kernel for NoTPAttention — fp8 q/k projection variant.

(dense transformer block:
fused QKV projection -> multi-head attention -> output projection).

Sharding (8 NeuronCores): core c handles batch b = c // 4 and the 4 heads
g = 4*(c % 4) .. 4*(c % 4)+3 (head-parallel tensor parallelism).  Each core
computes its heads' partial out-projection [S, H] in bf16; the host sums the
4 partials per batch in fp32 and adds the (folded) biases.

Numerics: all matmuls run in bf16 with fp32 PSUM accumulation.  Softmax is
computed without max-subtraction (scores are bounded, |s| < ~3.5) with the
normalization deferred to the attention *output*:
    attnT[d, q] = (sum_k v[k, d] * exp(sT[k, q])) / z[q],  z = sum_k exp
The denominator is NOT a full ones-matmul over every e-tile (that would cost
as much tensor time as the pv matmul).  Instead the DVE pre-reduces the 16
key-tiles of e elementwise (a 6-instruction bf16 add-ladder, cheap on the
otherwise idle vector engine), and a SINGLE ones-matmul per chunk
partition-reduces the [128, QC] ladder result, landing z already broadcast
across partitions.  This cuts the z tensor cost 16x (~51us/core).
The v-bias is dropped in-kernel: after normalization it contributes exactly
b_v to every row, so the host folds w_out @ b_v into the output bias.

Layout notes: qT/kT/attnT live as [128 (head-dim), head, seq] so every
matmul contracts over a full 128-partition tile with no transposes anywhere.
The qkv weights share SBUF slots with the attention exp-buffers (tag "e"):
they are dead once the projections finish, exactly when the exp buffers
start rotating.  Attention runs a depth-2 software pipeline (st/exp two
chunks ahead of pv) so the scalar engine's exp throughput (~8.7us/chunk vs
~7.0us of tensor work) never stalls the PE.  Startup DMAs are sliced per
4-ht block, interleaving wq and x so the first q-matmul starts after ~1MB
of traffic instead of 4MB.
"""

import numpy as np
import ml_dtypes

B, S, H = 2, 2048, 2048
NH, HD = 16, 128
P = 128
HT = H // P            # 16 hidden-dim tiles
G = 4                  # heads per core
GH = G * HD            # 512: head-group width per core
SCALE = 1.0 / float(np.sqrt(HD))
WS = 128.0             # fp8 q/k weight pre-scale
N_CORES = 8
XC = 512               # phase-1 x streaming chunk (s elements)
QC = 512               # attention query chunk
KT = S // P            # 16 key tiles

_CACHE = {}


def _build():
    import concourse.mybir as mybir
    import concourse.tile as tile
    from concourse import bacc

    dt = mybir.dt
    Alu = mybir.AluOpType
    Act = mybir.ActivationFunctionType

    nc = bacc.Bacc("TRN2", target_bir_lowering=False, debug=False,
                   enable_asserts=False)

    xt_d = nc.dram_tensor("xt", [H, S], dt.bfloat16, kind="ExternalInput").ap()
    xt8_d = nc.dram_tensor("xt8", [H, S], dt.float8e4,
                           kind="ExternalInput").ap()
    # q/k weights are fp8 (pre-scaled x128 on host), head-major so each
    # head's block is contiguous in DRAM.
    wqt_d = nc.dram_tensor("wqt8", [G * H, HD], dt.float8e4,
                           kind="ExternalInput").ap()
    wkt_d = nc.dram_tensor("wkt8", [G * H, HD], dt.float8e4,
                           kind="ExternalInput").ap()
    wvt_d = nc.dram_tensor("wvt", [H, GH], dt.bfloat16, kind="ExternalInput").ap()
    bqs_d = nc.dram_tensor("bqs", [P, G], dt.float32, kind="ExternalInput").ap()
    bk_d = nc.dram_tensor("bk", [P, G], dt.float32, kind="ExternalInput").ap()
    wot_d = nc.dram_tensor("wot", [GH, H], dt.bfloat16, kind="ExternalInput").ap()
    out_d = nc.dram_tensor("partial", [S, H], dt.bfloat16,
                           kind="ExternalOutput").ap()

    xt_r = xt_d.rearrange("(ht p) s -> p ht s", p=P)      # [128, 16, 2048]
    xt8_r = xt8_d.rearrange("(ht p) s -> p ht s", p=P)
    wqt_r = wqt_d.rearrange("(g ht p) d -> p g ht d", g=G, p=P)  # [128,4,16,128]
    wkt_r = wkt_d.rearrange("(g ht p) d -> p g ht d", g=G, p=P)
    wvt_r = wvt_d.rearrange("(ht p) o -> p ht o", p=P)
    wot_r = wot_d.rearrange("(g p) o -> p g o", p=P)      # [128, 4, 2048]

    NXC = S // XC      # 4
    NQC = S // QC      # 4

    with tile.TileContext(nc) as tc:
        with (
            tc.tile_pool(name="consts", bufs=1) as consts,
            tc.tile_pool(name="wpool", bufs=1) as wpool,
            tc.tile_pool(name="xpool", bufs=2) as xpool,
            tc.tile_pool(name="big", bufs=1) as big,
            tc.tile_pool(name="epool", bufs=4) as epool,
            tc.tile_pool(name="small", bufs=2) as small,
            tc.tile_pool(name="psum", bufs=2, space="PSUM") as psum,
        ):
            # --- startup DMAs, critical-path first.  wq slices issue from
            # the Sync queue while the x slices issue in parallel from the
            # (otherwise idle) GpSimd queue: DMA issue instructions cost
            # ~0.7-2us each, so serializing them on one engine delays the
            # first matmul.  First pieces are 2-ht (256KB) so the first
            # q-matmul group (which consumes ht ascending) starts early. ---
            # packed fp8 q+k weights: cols 0:GH = q heads, GH:2GH = k heads
            w8_sb = epool.tile([P, HT, 2 * GH], dt.float8e4, tag="e",
                               name="w8_sb")
            xt80_sb = xpool.tile([P, HT, XC], dt.float8e4, tag="xt8",
                                 name="xt80_sb")
            for h in range(G):
                nc.sync.dma_start(w8_sb[:, :, h * HD:(h + 1) * HD],
                                  wqt_r[:, h, :, :])
            for lo, hi in ((0, 8), (8, 16)):
                sl = slice(lo, hi)
                nc.gpsimd.dma_start(xt80_sb[:, sl, :], xt8_r[:, sl, 0:XC])
            # x (bf16) for the v projection; v stays bf16 for accuracy.
            # Ordering matters: k weights and the v-pass inputs (xt0, wv)
            # must land before their first consumers (~7us and ~14us after
            # the q groups start), so they interleave here.
            xt0_sb = xpool.tile([P, HT, XC], dt.bfloat16, tag="xt",
                                name="xt0_sb")
            nc.sync.dma_start(xt0_sb[:, 0:8, :], xt_r[:, 0:8, 0:XC])
            for h in range(G):
                nc.sync.dma_start(w8_sb[:, :, GH + h * HD:GH + (h + 1) * HD],
                                  wkt_r[:, h, :, :])
            nc.sync.dma_start(xt0_sb[:, 8:16, :], xt_r[:, 8:16, 0:XC])
            wv_sb = epool.tile([P, HT, GH], dt.bfloat16, tag="e", name="wv_sb")
            nc.sync.dma_start(wv_sb[:, 0:8, :], wvt_r[:, 0:8, :])
            nc.sync.dma_start(wv_sb[:, 8:16, :], wvt_r[:, 8:16, :])
            bqs_sb = consts.tile([P, G], dt.float32)
            nc.gpsimd.dma_start(bqs_sb[:], bqs_d)
            bk_sb = consts.tile([P, G], dt.float32)
            nc.gpsimd.dma_start(bk_sb[:], bk_d)
            ones_sb = consts.tile([P, P], dt.bfloat16)
            nc.vector.memset(ones_sb[:], 1.0)
            # PE warmup: TRN2 ramps the PE clock (0.65 -> 2.4 GHz) only while
            # the array is busy; throwaway matmuls during the startup DMA
            # wait mean the first real matmuls run at speed.
            for _ in range(16):
                wps = psum.tile([P, 512], dt.float32, tag="mm")
                nc.tensor.matmul(wps[:, 0:P], ones_sb[:], ones_sb[:],
                                 start=True, stop=True)

            qt_sb = big.tile([P, G, S], dt.bfloat16)   # q^T, scale+bias applied
            kt_sb = big.tile([P, G, S], dt.bfloat16)   # k^T, bias applied
            v_sb = big.tile([P, KT, GH], dt.bfloat16)  # v natural [s, o]
            at_tiles = {}                              # attn out^T, ring per qc

            # ---------------- Phase 1: QKV projections ----------------
            DR = mybir.MatmulPerfMode.DoubleRow

            def emit_v(xc, xt_sb):
                for sv in range(XC // P):
                    sm = xc * (XC // P) + sv
                    psv = psum.tile([P, 512], dt.float32, tag="mm")
                    for ht in range(HT):
                        nc.tensor.matmul(psv,
                                         xt_sb[:, ht, sv * P:(sv + 1) * P],
                                         wv_sb[:, ht, :],
                                         start=(ht == 0), stop=(ht == HT - 1))
                    nc.vector.tensor_copy(out=v_sb[:, sm, :], in_=psv)

            # v(xc) is emitted one iteration late (after q/k of xc+1): the
            # bf16 x-stream and wv then have a full extra xc window to land,
            # keeping the startup critical path to just w8 + the fp8 x chunk.
            vjob = None
            for xc in range(NXC):
                if xc == 0:
                    xt8_sb = xt80_sb
                    xt_sb = xt0_sb
                else:
                    xt8_sb = xpool.tile([P, HT, XC], dt.float8e4, tag="xt8",
                                        name="xt8_sb")
                    nc.gpsimd.dma_start(xt8_sb[:],
                                        xt8_r[:, :, xc * XC:(xc + 1) * XC])
                    xt_sb = xpool.tile([P, HT, XC], dt.bfloat16, tag="xt",
                                       name="xt_sb")
                    nc.sync.dma_start(xt_sb[:], xt_r[:, :, xc * XC:(xc + 1) * XC])
                sl = slice(xc * XC, (xc + 1) * XC)
                for h in range(G):
                    psq = psum.tile([P, 512], dt.float32, tag="mm")
                    for hp in range(HT // 2):
                        nc.tensor.matmul(psq,
                                         w8_sb[:, 2 * hp:2 * hp + 2,
                                               h * HD:(h + 1) * HD],
                                         xt8_sb[:, 2 * hp:2 * hp + 2, :],
                                         start=(hp == 0), stop=(hp == HT // 2 - 1),
                                         perf_mode=DR)
                    nc.vector.tensor_scalar(qt_sb[:, h, sl], psq,
                                            SCALE / WS, bqs_sb[:, h:h + 1],
                                            Alu.mult, Alu.add)
                for h in range(G):
                    psk = psum.tile([P, 512], dt.float32, tag="mm")
                    for hp in range(HT // 2):
                        nc.tensor.matmul(psk,
                                         w8_sb[:, 2 * hp:2 * hp + 2,
                                               GH + h * HD:GH + (h + 1) * HD],
                                         xt8_sb[:, 2 * hp:2 * hp + 2, :],
                                         start=(hp == 0), stop=(hp == HT // 2 - 1),
                                         perf_mode=DR)
                    nc.vector.tensor_scalar(kt_sb[:, h, sl], psk,
                                            1.0 / WS, bk_sb[:, h:h + 1],
                                            Alu.mult, Alu.add)
                if vjob is not None:
                    emit_v(*vjob)
                vjob = (xc, xt_sb)
            emit_v(*vjob)

            # out-proj weights: needed only from the first proj (~mid-kernel)
            wo_sb = wpool.tile([P, G, H], dt.bfloat16)
            nc.sync.dma_start(wo_sb[:], wot_r)

            # -------- Phase 2+3: attention + out-proj (sw-pipelined) --------
            def emit_st_exp(h, qc, pvjob=None):
                # ST^T = k^T.T @ q^T per 128-key tile; exp on ACT in 2-bank
                # batches (halves the per-ACTIVATE overhead).  The previous
                # chunk's pv matmuls (whose exp inputs are long done) are
                # interleaved between st pairs so the tensor engine never
                # stalls on the ACT-paced st psum ring.
                e_sb = epool.tile([P, KT, QC], dt.bfloat16, tag="e",
                                  name="e_sb")
                for km in range(0, KT, 2):
                    ps = psum.tile([P, 2, QC], dt.float32, tag="st")
                    for j in range(2):
                        nc.tensor.matmul(ps[:, j, :],
                                         kt_sb[:, h, (km + j) * P:(km + j + 1) * P],
                                         qt_sb[:, h, qc * QC:(qc + 1) * QC],
                                         start=True, stop=True)
                    if pvjob is not None:
                        pv, ph, pe = pvjob
                        for j in range(2):
                            nc.tensor.matmul(pv,
                                             v_sb[:, km + j, ph * HD:(ph + 1) * HD],
                                             pe[:, km + j, :],
                                             start=(km + j == 0),
                                             stop=(km + j == KT - 1))
                    nc.scalar.activation(e_sb[:, km:km + 2, :], ps, Act.Exp)
                return e_sb

            def alloc_pv(h, qc):
                if h == 0:
                    at_tiles[qc] = small.tile([P, G, QC], dt.bfloat16,
                                              tag="at", bufs=2, name="at_t")
                pv = psum.tile([P, QC], dt.float32, tag="pv", bufs=2)
                return pv

            def emit_pv_plain(pv, h, e_sb):
                for km in range(KT):
                    nc.tensor.matmul(pv, v_sb[:, km, h * HD:(h + 1) * HD],
                                     e_sb[:, km, :],
                                     start=(km == 0), stop=(km == KT - 1))

            def emit_z_norm(pv, h, qc, e_sb):
                # z: DVE add-ladder over the 16 key-tiles of e (bf16, packed
                # SBUF operands -> fast DVE mode), then ONE ones-matmul to
                # partition-reduce, landing z broadcast across partitions.
                lA = small.tile([P, 4, QC], dt.bfloat16, tag="l4")
                nc.vector.tensor_add(out=lA[:], in0=e_sb[:, 0:4, :],
                                     in1=e_sb[:, 4:8, :])
                lB = small.tile([P, 4, QC], dt.bfloat16, tag="l4")
                nc.vector.tensor_add(out=lB[:], in0=e_sb[:, 8:12, :],
                                     in1=e_sb[:, 12:16, :])
                lC = small.tile([P, 2, QC], dt.bfloat16, tag="l2")
                nc.vector.tensor_add(out=lC[:], in0=lA[:, 0:2, :],
                                     in1=lA[:, 2:4, :])
                lD = small.tile([P, 2, QC], dt.bfloat16, tag="l2")
                nc.vector.tensor_add(out=lD[:], in0=lB[:, 0:2, :],
                                     in1=lB[:, 2:4, :])
                lE = small.tile([P, 2, QC], dt.bfloat16, tag="le", bufs=1)
                nc.vector.tensor_add(out=lE[:], in0=lC[:], in1=lD[:])
                esum = small.tile([P, QC], dt.bfloat16, tag="es", bufs=1)
                nc.vector.tensor_add(out=esum[:], in0=lE[:, 0, :],
                                     in1=lE[:, 1, :])
                # z shares the "mm" psum ring (its own bank would push the
                # total over 8 now that pv is double-buffered)
                z = psum.tile([P, QC], dt.float32, tag="mm")
                nc.tensor.matmul(z, ones_sb[:], esum[:], start=True, stop=True)
                zi = small.tile([P, QC], dt.float32, tag="zi", bufs=1)
                nc.vector.reciprocal_approx_fast(out=zi[:], in_=z)
                nc.vector.tensor_mul(out=at_tiles[qc][:, h, :],
                                     in0=pv, in1=zi[:])

            def emit_proj(qc, last=False):
                for sv in range(QC // P):
                    sm = qc * (QC // P) + sv
                    ob = None
                    for oc in range(H // 512):
                        pp = psum.tile([P, 512], dt.float32, tag="mm")
                        for g in range(G):
                            nc.tensor.matmul(pp,
                                             at_tiles[qc][:, g, sv * P:(sv + 1) * P],
                                             wo_sb[:, g, oc * 512:(oc + 1) * 512],
                                             start=(g == 0), stop=(g == G - 1))
                        if oc % 2 == 0:
                            ob = small.tile([P, 2, 512], dt.bfloat16, tag="ob",
                                            bufs=2)
                        # in the final group, split the drain copies across
                        # DVE and ACT so the tail isn't serialized on one
                        # engine (Copy is in every ACT table set: no reload;
                        # GpSimd cannot read PSUM on TRN2).
                        if last and oc % 2 == 1:
                            nc.scalar.copy(ob[:, oc % 2, :], pp)
                        else:
                            nc.vector.tensor_copy(out=ob[:, oc % 2, :], in_=pp)
                        if oc % 2 == 1:
                            nc.sync.dma_start(
                                out_d[sm * P:(sm + 1) * P,
                                      (oc - 1) * 512:(oc + 1) * 512],
                                ob[:])

            chunks = [(h, qc) for qc in range(NQC) for h in range(G)]
            emitted = []
            for i, (h, qc) in enumerate(chunks):
                if i >= 2:
                    ph, pqc, pe = emitted[i - 2]
                    pv = alloc_pv(ph, pqc)
                    e = emit_st_exp(h, qc, pvjob=(pv, ph, pe))
                    emitted.append((h, qc, e))
                    emit_z_norm(pv, ph, pqc, pe)
                    if ph == G - 1:
                        emit_proj(pqc)
                else:
                    e = emit_st_exp(h, qc)
                    emitted.append((h, qc, e))
            for i in (len(chunks) - 2, len(chunks) - 1):
                ph, pqc, pe = emitted[i]
                pv = alloc_pv(ph, pqc)
                emit_pv_plain(pv, ph, pe)
                emit_z_norm(pv, ph, pqc, pe)
                if ph == G - 1:
                    emit_proj(pqc, last=(i == len(chunks) - 1))

    nc.compile()
    return nc


def _get_nc():
    if "nc" not in _CACHE:
        _CACHE["nc"] = _build()
    return _CACHE["nc"]


def _head_major(wt):
    # [H, GH] -> head-major [G*H, HD]
    return np.ascontiguousarray(
        wt.reshape(H, G, HD).transpose(1, 0, 2).reshape(G * H, HD))


def _make_in_maps(x, w_qkv, b_qkv, w_out):
    bf = ml_dtypes.bfloat16
    f8 = ml_dtypes.float8_e4m3
    f32 = np.float32
    in_maps = []
    for c in range(N_CORES):
        b = c // 4
        g = c % 4
        lo = GH * g
        hi = GH * (g + 1)
        xt = np.ascontiguousarray(x[b].T)
        xt8 = xt.astype(f8)
        wqt8 = _head_major(w_qkv[lo:hi, :].T * WS).astype(f8)
        wkt8 = _head_major(w_qkv[H + lo:H + hi, :].T * WS).astype(f8)
        wvt = np.ascontiguousarray(w_qkv[2 * H + lo:2 * H + hi, :].T).astype(bf)
        bqs = np.ascontiguousarray(
            (b_qkv[lo:hi] * SCALE).astype(f32).reshape(G, P).T)
        bk = np.ascontiguousarray(
            b_qkv[H + lo:H + hi].astype(f32).reshape(G, P).T)
        wot = np.ascontiguousarray(w_out[:, lo:hi].T).astype(bf)
        in_maps.append({"xt": xt.astype(bf), "xt8": xt8, "wqt8": wqt8,
                        "wkt8": wkt8, "wvt": wvt,
                        "bqs": bqs, "bk": bk, "wot": wot})
    return in_maps


def kernel(x, w_qkv, b_qkv, w_out, b_out):
    import os
    import sys

    x = np.asarray(x, dtype=np.float32)
    w_qkv = np.asarray(w_qkv, dtype=np.float32)
    b_qkv = np.asarray(b_qkv, dtype=np.float32)
    w_out = np.asarray(w_out, dtype=np.float32)
    b_out = np.asarray(b_out, dtype=np.float32)

    from concourse.bass_utils import run_bass_kernel_spmd

    # NTFF tracing under axon needs the antenv.axon_hooks shim (test.py
    # installs it); without it a stray BASS_TRACE=1 in the environment would
    # crash the run — disable tracing in that case.
    if "antenv.axon_hooks" not in sys.modules:
        os.environ["BASS_NEVER_TRACE"] = "1"

    nc = _get_nc()
    in_maps = _make_in_maps(x, w_qkv, b_qkv, w_out)
    res = run_bass_kernel_spmd(nc, in_maps, core_ids=list(range(N_CORES)))
    _CACHE["last_results"] = res
    partials = [r["partial"] for r in res.results]

    bv = b_qkv[2 * H:3 * H]
    bias = b_out + w_out @ bv          # folded v-bias contribution
    out = np.empty((B, S, H), np.float32)
    for b in range(B):
        acc = partials[4 * b].astype(np.float32)
        for g in range(1, 4):
            acc += partials[4 * b + g].astype(np.float32)
        out[b] = acc + bias
    return out
